# revision 14
# baseline (speedup 1.0000x reference)
"""Trainium2 Bass kernel for nn_FPthenAllConcatDecoder256 (PointNet++ FP decoder).

kernel(**inputs) takes FULL unsharded inputs (p1..p5, f1..f5, params) and
returns the FULL [8, 32, 8192] float32 output. Batch-parallel across 8
NeuronCores (1 batch element per core, one SPMD program).

Per-core algorithm:
  - three_nn selections computed EXACTLY (bitwise-identical fp32 arithmetic to
    the jax reference: d2 = ((dx^2+dy^2)+dz^2), top-3 by value, ties broken by
    lower index) using DVE ops on replicated source-coordinate tables + the
    hardware Max8/MaxIndex instructions (duplicate-value semantics match
    lax.top_k tie-breaking).
  - Feature interpolation via SDMA row gathers (dma_gather) from DRAM tables
    of [src, C] feature rows; weighted sum on DVE with per-partition scalars.
  - 1x1 convs as channels-on-partition PE matmuls, folded BN+ReLU epilogue on
    the scalar engine; PE transposes convert [pts, C] gathers to [C, pts].
"""

import sys

sys.path.insert(0, "/opt/trn_rl_repo")

import numpy as np
from contextlib import ExitStack

import concourse.bass as bass
import concourse.bacc as bacc
import concourse.tile as tile
from concourse import mybir
from concourse.bass_utils import run_bass_kernel_spmd
from concourse.library_config import mlp as mlp_lib

F32 = mybir.dt.float32
U16 = mybir.dt.uint16
I16 = mybir.dt.int16
AX = mybir.AxisListType
ALU = mybir.AluOpType
ACTF = mybir.ActivationFunctionType

B = 8
N1, N2, N3, N4, N5 = 8192, 2048, 512, 128, 32


# --------------------------- host-side packing -----------------------------

def _pack_dst(p):
    N = p.shape[0]
    T = N // 128
    return np.ascontiguousarray(
        p.reshape(T, 128, 3).transpose(1, 0, 2).reshape(128, T * 3)
    ).astype(np.float32)


def _pack_rep(p):
    M = p.shape[0]
    one = np.ascontiguousarray(p.T.reshape(1, 3 * M)).astype(np.float32)
    return np.broadcast_to(one, (128, 3 * M)).copy()


def _pack_w(w, perm=None):
    wt = np.asarray(w, np.float32).T  # [Cin, Cout]
    if perm is not None:
        wt = wt[perm]
    cin, cout = wt.shape
    nkt = (cin + 127) // 128
    out = np.zeros((128, nkt * cout), np.float32)
    for k in range(nkt):
        blk = wt[k * 128 : (k + 1) * 128]
        out[: blk.shape[0], k * cout : k * cout + cout] = blk
    return out


def _pack_sb(s, b):
    s = np.asarray(s, np.float32)
    b = np.asarray(b, np.float32)
    cout = s.shape[0]
    nco = (cout + 127) // 128
    out = np.zeros((128, 2 * nco), np.float32)
    for c in range(nco):
        lo, hi = c * 128, min((c + 1) * 128, cout)
        out[: hi - lo, 2 * c] = s[lo:hi]
        out[: hi - lo, 2 * c + 1] = b[lo:hi]
    return out


def host_prep(inputs, b):
    f32 = np.float32
    p = {i: np.asarray(inputs["p%d" % i][b], f32) for i in range(1, 6)}
    f = {i: np.asarray(inputs["f%d" % i][b], f32) for i in range(1, 6)}
    prm = inputs["params"]

    m = {}
    m["p1P"] = _pack_dst(p[1])
    m["p2P"] = _pack_dst(p[2])
    m["p3P"] = _pack_dst(p[3])
    m["p4P"] = _pack_dst(p[4])
    m["p2rep"] = _pack_rep(p[2])
    m["p3rep"] = _pack_rep(p[3])
    m["p4rep"] = _pack_rep(p[4])
    m["p5rep"] = _pack_rep(p[5])
    for i in range(1, 4):
        m["f%d" % i] = np.ascontiguousarray(f[i])
    m["f4"] = np.ascontiguousarray(
        f[4].reshape(2, 128, 128).transpose(1, 0, 2).reshape(128, 256))
    m["f5"] = np.ascontiguousarray(
        f[5].reshape(4, 128, 32).transpose(1, 0, 2).reshape(128, 128))
    m["Tf5"] = np.ascontiguousarray(f[5].T)  # [32, 512]
    m["ident"] = np.eye(128, dtype=f32)

    def dec_perm(c_skip, c_interp):
        return np.concatenate(
            [np.arange(c_skip, c_skip + c_interp), np.arange(c_skip)]
        )

    for nm, cs, cint in (("dec4", 256, 512), ("dec3", 128, 256),
                         ("dec2", 64, 128), ("dec1", 32, 64)):
        w, s, bb = prm[nm][0]
        m["W%sa" % nm] = _pack_w(w, dec_perm(cs, cint))
        m["S%sa" % nm] = _pack_sb(s, bb)
        w, s, bb = prm[nm][1]
        m["W%sb" % nm] = _pack_w(w)
        m["S%sb" % nm] = _pack_sb(s, bb)
    for nm in ("t5", "t4", "t3", "t2"):
        w, s, bb = prm[nm]
        m["W" + nm] = _pack_w(w)
        m["S" + nm] = _pack_sb(s, bb)
    perm = np.concatenate(
        [np.arange(224, 480), np.arange(480, 736), np.arange(96, 224),
         np.arange(32, 96), np.arange(0, 32)]
    )
    w, s, bb = prm["fuse"][0]
    m["Wfusea"] = _pack_w(w, perm)
    m["Sfusea"] = _pack_sb(s, bb)
    w, s, bb = prm["fuse"][1]
    m["Wfuseb"] = _pack_w(w)
    m["Sfuseb"] = _pack_sb(s, bb)
    return m


INPUT_SPECS = {
    "p1P": (128, 64 * 3), "p2P": (128, 16 * 3), "p3P": (128, 4 * 3),
    "p4P": (128, 1 * 3),
    "p2rep": (128, 3 * N2), "p3rep": (128, 3 * N3), "p4rep": (128, 3 * N4),
    "p5rep": (128, 3 * N5),
    "f1": (32, N1), "f2": (64, N2), "f3": (128, N3), "f4": (128, 2 * N4),
    "f5": (128, 4 * N5),
    "Tf5": (32, 512), "ident": (128, 128),
    "Wdec4a": (128, 6 * 256), "Sdec4a": (128, 4),
    "Wdec4b": (128, 2 * 256), "Sdec4b": (128, 4),
    "Wdec3a": (128, 3 * 128), "Sdec3a": (128, 2),
    "Wdec3b": (128, 128), "Sdec3b": (128, 2),
    "Wdec2a": (128, 2 * 64), "Sdec2a": (128, 2),
    "Wdec2b": (128, 64), "Sdec2b": (128, 2),
    "Wdec1a": (128, 32), "Sdec1a": (128, 2),
    "Wdec1b": (128, 32), "Sdec1b": (128, 2),
    "Wt5": (128, 4 * 256), "St5": (128, 4),
    "Wt4": (128, 2 * 256), "St4": (128, 4),
    "Wt3": (128, 128), "St3": (128, 2),
    "Wt2": (128, 64), "St2": (128, 2),
    "Wfusea": (128, 6 * 64), "Sfusea": (128, 2),
    "Wfuseb": (128, 32), "Sfuseb": (128, 2),
}


def build_kernel():
    nc = bacc.Bacc("TRN2", target_bir_lowering=False, debug=False)
    ins = {}
    for name, shape in INPUT_SPECS.items():
        ins[name] = nc.dram_tensor(name, list(shape), F32, kind="ExternalInput")
    out_h = nc.dram_tensor("out", [32, N1], F32, kind="ExternalOutput")

    with tile.TileContext(nc) as tc:
        with ExitStack() as ctx:
            _emit(ctx, tc, ins, out_h)

    nc.compile()
    return nc


def _emit(ctx, tc, ins, out_h):
    nc = tc.nc
    nc.gpsimd.load_library(mlp_lib)

    const = ctx.enter_context(tc.tile_pool(name="const", bufs=1))
    sel_pool = ctx.enter_context(tc.tile_pool(name="sel", bufs=1))
    small = ctx.enter_context(tc.tile_pool(name="small", bufs=2))
    wpool = ctx.enter_context(tc.tile_pool(name="w", bufs=1))
    psum = ctx.enter_context(tc.tile_pool(name="ps", bufs=2, space="PSUM"))
    feat = ctx.enter_context(tc.tile_pool(name="feat", bufs=1))
    gath = ctx.enter_context(tc.tile_pool(name="gath", bufs=1))
    fcatp = ctx.enter_context(tc.tile_pool(name="fcat", bufs=2))
    dram = ctx.enter_context(tc.tile_pool(name="drm", bufs=1, space="DRAM"))

    # internal DRAM tables + idx staging (pool tiles => dependency-tracked)
    Tf4n = dram.tile([N4, 256], F32, tag="Tf4n")
    Tg4 = dram.tile([N4, 256], F32, tag="Tg4")
    Tf3n = dram.tile([N3, 128], F32, tag="Tf3n")
    Tg3 = dram.tile([N3, 128], F32, tag="Tg3")
    Tf2 = dram.tile([N2, 128], F32, tag="Tf2")
    Tg5 = dram.tile([N5, 256], F32, tag="Tg5")
    stg = {
        "s45": dram.tile([N4, 3], U16, tag="stg45", name="stg45"),
        "s34": dram.tile([N3, 3], U16, tag="stg34", name="stg34"),
        "s23": dram.tile([N2, 3], U16, tag="stg23", name="stg23"),
        "L2": dram.tile([N1, 3], U16, tag="stgL2", name="stgL2"),
        "L3": dram.tile([N1, 3], U16, tag="stgL3", name="stgL3"),
        "L4": dram.tile([N1, 3], U16, tag="stgL4", name="stgL4"),
        "L5": dram.tile([N1, 3], U16, tag="stgL5", name="stgL5"),
    }

    def load(name, pool=const):
        h = ins[name]
        t = pool.tile(list(h.shape), F32, tag=name, name="ld_"+name)
        nc.sync.dma_start(out=t[:], in_=h[:])
        return t

    ident = load("ident")
    pP = {1: load("p1P"), 2: load("p2P"), 3: load("p3P"), 4: load("p4P")}
    rep = {2: load("p2rep"), 3: load("p3rep"), 4: load("p4rep"),
           5: load("p5rep")}
    fin = {i: load("f%d" % i) for i in range(2, 6)}
    W = {k: load(k, wpool) for k in ins if k[0] in "WS" and k != "Tf5"}

    WS = {}
    for key, T in (("s45", 1), ("s34", 4), ("s23", 16), ("L2", 64),
                   ("L3", 64), ("L4", 64), ("L5", 64)):
        WS[key] = const.tile([128, T * 3], F32, tag="ws" + key, name="ws" + key)

    # ---- exact three_nn selection --------------------------------------
    def select_level(key, dstP, repT, M, T):
        for t in range(T):
            negd2 = sel_pool.tile([128, M], F32, tag="negd2")
            dx = sel_pool.tile([128, M], F32, tag="dx")
            sq = sel_pool.tile([128, M], F32, tag="sq")
            px = dstP[:, t * 3 : t * 3 + 1]
            py = dstP[:, t * 3 + 1 : t * 3 + 2]
            pz = dstP[:, t * 3 + 2 : t * 3 + 3]
            nc.vector.tensor_scalar_sub(dx[:], repT[:, 0:M], px)
            nc.vector.tensor_tensor(negd2[:], dx[:], dx[:], ALU.mult)
            nc.vector.tensor_scalar_sub(dx[:], repT[:, M : 2 * M], py)
            nc.vector.tensor_tensor(sq[:], dx[:], dx[:], ALU.mult)
            nc.vector.scalar_tensor_tensor(
                negd2[:], negd2[:], -1.0, sq[:], op0=ALU.mult, op1=ALU.subtract
            )
            nc.vector.tensor_scalar_sub(dx[:], repT[:, 2 * M : 3 * M], pz)
            nc.vector.tensor_tensor(sq[:], dx[:], dx[:], ALU.mult)
            nc.vector.tensor_tensor(negd2[:], negd2[:], sq[:], ALU.subtract)

            v8 = small.tile([128, 8], F32, tag="v8")
            i8 = small.tile([128, 8], U16, tag="i8")
            nc.vector.max(v8[:], negd2[:])
            nc.vector.max_index(i8[:], v8[:], negd2[:])

            d3 = small.tile([128, 3], F32, tag="d3")
            rc = small.tile([128, 3], F32, tag="rc")
            ssum = small.tile([128, 1], F32, tag="ssum")
            nc.scalar.activation(d3[:], v8[:, 0:3], ACTF.Sqrt, scale=-1.0)
            nc.vector.tensor_scalar_add(d3[:], d3[:], 1e-8)
            nc.vector.reciprocal(rc[:], d3[:])
            nc.vector.tensor_reduce(ssum[:], rc[:], axis=AX.X, op=ALU.add)
            rsum = small.tile([128, 1], F32, tag="rsum")
            nc.vector.reciprocal(rsum[:], ssum[:])
            nc.vector.tensor_scalar_mul(
                WS[key][:, t * 3 : t * 3 + 3], rc[:], rsum[:]
            )
            nc.sync.dma_start(
                out=stg[key][t * 128 : (t + 1) * 128, :], in_=i8[:, 0:3]
            )

    def load_wrapped(key, N):
        wt = const.tile([128, 3 * N // 16], U16, tag="wrap" + key, name="wrap" + key)
        src = stg[key][:, :].rearrange("(q s) j -> s j q", s=16)
        nc.sync.dma_start(
            out=wt[0:16, :].rearrange("s (j q) -> s j q", j=3), in_=src
        )
        for grp in range(1, 8):
            nc.sync.dma_start(
                out=wt[grp * 16 : (grp + 1) * 16, :], in_=wt[0:16, :]
            )
        return wt

    def gather_j(table_ap, C, wrapped, N, j, t0, nt, tg="qA"):
        g = gath.tile([128, nt, C], F32, tag="%s_%d" % (tg, j),
                      name="%s_%d" % (tg, j))
        col0 = (j * N + t0 * 128) // 16
        nc.gpsimd.dma_gather(
            g[:], table_ap,
            wrapped[:, col0 : col0 + nt * 128 // 16].bitcast(I16),
            nt * 128, nt * 128, C,
        )
        return g

    def wsum(gs, key, t, t0, C):
        a = small.tile([128, C], F32, tag="ws_a%d" % C)
        b = small.tile([128, C], F32, tag="ws_b%d" % C)
        w0 = WS[key][:, t * 3 : t * 3 + 1]
        w1 = WS[key][:, t * 3 + 1 : t * 3 + 2]
        w2 = WS[key][:, t * 3 + 2 : t * 3 + 3]
        ti = t - t0
        nc.vector.tensor_scalar_mul(a[:], gs[0][:, ti, :], w0)
        nc.vector.scalar_tensor_tensor(
            b[:], gs[1][:, ti, :], w1, a[:], op0=ALU.mult, op1=ALU.add
        )
        nc.vector.scalar_tensor_tensor(
            a[:], gs[2][:, ti, :], w2, b[:], op0=ALU.mult, op1=ALU.add
        )
        return a

    def transpose_to(dst_ap, src_ap, C):
        pt = psum.tile([128, 128], F32, tag="tr")
        nc.tensor.transpose(pt[0:C, 0:128], src_ap, ident[:])
        nc.scalar.copy(dst_ap, pt[0:C, 0:128])

    def conv(xs, Wn, Sn, Cout, N, out_into=None, pbase=0):
        nkt = len(xs)
        outs = []
        for co in range((Cout + 127) // 128):
            com = min(128, Cout - co * 128)
            pt = psum.tile([128, N], F32, tag="cv%d" % N)
            for k, (x_ap, rows) in enumerate(xs):
                c0 = k * Cout + co * 128
                nc.tensor.matmul(
                    pt[pbase : pbase + com, 0:N],
                    W[Wn][0:rows, c0 : c0 + com], x_ap,
                    start=(k == 0), stop=(k == nkt - 1),
                    tile_position=(0, pbase) if pbase else None,
                )
            if out_into is not None:
                o_ap = out_into[co]
            else:
                ot = feat.tile([128, N], F32, tag="cv%s_%d" % (Wn, co), name="cv%s_%d" % (Wn, co))
                outs.append(ot)
                o_ap = ot[0:com, 0:N]
            nc.scalar.activation(
                o_ap, pt[pbase : pbase + com, 0:N], ACTF.Relu,
                bias=W[Sn][0:com, 2 * co + 1 : 2 * co + 2],
                scale=W[Sn][0:com, 2 * co : 2 * co + 1],
            )
        return outs

    def write_table(table, ftiles, Npts, col0=0):
        for pc in range(Npts // 128):
            coff = col0
            for (ft, rows) in ftiles:
                pt = psum.tile([128, 128], F32, tag="tr")
                sb = small.tile([128, 128], F32, tag="trwsb")
                nc.tensor.transpose(
                    pt[0:128, 0:rows], ft[0:rows, pc * 128 : (pc + 1) * 128],
                    ident[0:rows, 0:rows],
                )
                nc.scalar.copy(sb[0:128, 0:rows], pt[0:128, 0:rows])
                nc.sync.dma_start(
                    out=table[pc * 128 : (pc + 1) * 128, coff : coff + rows],
                    in_=sb[0:128, 0:rows],
                )
                coff += rows

    # ------------------------------ Phase A -----------------------------
    # A1: p4<-p5 + dec4
    select_level("s45", pP[4], rep[5], N5, 1)
    w45 = load_wrapped("s45", N4)
    g45 = [gather_j(ins["Tf5"][:, :], 512, w45, N4, j, 0, 1, "qD") for j in range(3)]
    i45 = wsum(g45, "s45", 0, 0, 512)
    d4in = feat.tile([128, 6 * 128], F32, tag="d4in")
    for k in range(4):
        transpose_to(
            d4in[:, k * 128 : (k + 1) * 128],
            i45[:, k * 128 : (k + 1) * 128], 128,
        )
    nc.scalar.copy(d4in[:, 4 * 128 : 5 * 128], fin[4][:, 0:128])
    nc.scalar.copy(d4in[:, 5 * 128 : 6 * 128], fin[4][:, 128:256])
    xs = [(d4in[:, k * 128 : (k + 1) * 128], 128) for k in range(6)]
    h4 = conv(xs, "Wdec4a", "Sdec4a", 256, 128)
    f4n = conv([(h4[0][:], 128), (h4[1][:], 128)], "Wdec4b", "Sdec4b", 256, 128)
    t4f = conv([(f4n[0][:], 128), (f4n[1][:], 128)], "Wt4", "St4", 256, 128)
    write_table(Tf4n, [(f4n[0], 128), (f4n[1], 128)], N4)
    write_table(Tg4, [(t4f[0], 128), (t4f[1], 128)], N4)

    # t5(f5) -> Tg5
    xs5 = [(fin[5][:, k * 32 : (k + 1) * 32], 128) for k in range(4)]
    t5f = conv(xs5, "Wt5", "St5", 256, 32)
    for ci in range(2):
        pt = psum.tile([128, 128], F32, tag="tr")
        sb = small.tile([128, 128], F32, tag="trwsb")
        nc.tensor.transpose(pt[0:32, 0:128], t5f[ci][0:128, 0:32], ident[:])
        nc.scalar.copy(sb[0:32, 0:128], pt[0:32, 0:128])
        nc.sync.dma_start(
            out=Tg5[0:32, ci * 128 : (ci + 1) * 128], in_=sb[0:32, 0:128]
        )

    # A2: p3<-p4 + dec3
    select_level("s34", pP[3], rep[4], N4, 4)
    w34 = load_wrapped("s34", N3)
    gs34 = [gather_j(Tf4n[:, :], 256, w34, N3, j, 0, 4, "qB") for j in range(3)]
    d3in = feat.tile([128, 3 * 512], F32, tag="d3in")
    for t in range(4):
        a = wsum(gs34, "s34", t, 0, 256)
        for k in range(2):
            transpose_to(
                d3in[:, k * 512 + t * 128 : k * 512 + (t + 1) * 128],
                a[:, k * 128 : (k + 1) * 128], 128,
            )
    nc.scalar.copy(d3in[:, 2 * 512 : 3 * 512], fin[3][:])
    xs = [(d3in[:, k * 512 : (k + 1) * 512], 128) for k in range(3)]
    h3 = conv(xs, "Wdec3a", "Sdec3a", 128, 512)
    f3n = conv([(h3[0][:], 128)], "Wdec3b", "Sdec3b", 128, 512)
    t3f = conv([(f3n[0][:], 128)], "Wt3", "St3", 128, 512)
    write_table(Tf3n, [(f3n[0], 128)], N3)
    write_table(Tg3, [(t3f[0], 128)], N3)

    # A3: p2<-p3 + dec2
    select_level("s23", pP[2], rep[3], N3, 16)
    w23 = load_wrapped("s23", N2)
    d2in = feat.tile([128, 512], F32, tag="d2in")
    for ch in range(4):
        f2nc = feat.tile([64, 512], F32, tag="f2nc", name="f2nc")
        t2fc = feat.tile([64, 512], F32, tag="t2fc", name="t2fc")
        gs23 = [gather_j(Tf3n[:, :], 128, w23, N2, j, ch * 4, 4, "qA")
                for j in range(3)]
        for t in range(4):
            a = wsum(gs23, "s23", ch * 4 + t, ch * 4, 128)
            transpose_to(
                d2in[:, t * 128 : (t + 1) * 128], a[:], 128
            )
        xs = [(d2in[:, 0:512], 128),
              (fin[2][0:64, ch * 512 : (ch + 1) * 512], 64)]
        h2 = conv(xs, "Wdec2a", "Sdec2a", 64, 512)
        conv([(h2[0][0:64, 0:512], 64)], "Wdec2b", "Sdec2b", 64, 512,
             out_into=[f2nc[0:64, 0:512]])
        conv([(f2nc[0:64, 0:512], 64)], "Wt2", "St2",
             64, 512, out_into=[t2fc[0:64, 0:512]])
        write_table(Tf2[ch * 512 : (ch + 1) * 512, :], [(f2nc, 64)], 512,
                    col0=0)
        write_table(Tf2[ch * 512 : (ch + 1) * 512, :], [(t2fc, 64)], 512,
                    col0=64)

    # A4: p1-level selections
    select_level("L2", pP[1], rep[2], N2, 64)
    select_level("L3", pP[1], rep[3], N3, 64)
    select_level("L4", pP[1], rep[4], N4, 64)
    select_level("L5", pP[1], rep[5], N5, 64)
    wL2 = load_wrapped("L2", N1)
    wL3 = load_wrapped("L3", N1)
    wL4 = load_wrapped("L4", N1)
    wL5 = load_wrapped("L5", N1)

    # ------------------------------ Phase B -----------------------------
    NT = 4
    for sc in range(64 // NT):
        t0 = sc * NT
        gL2 = [gather_j(Tf2[:, :], 128, wL2, N1, j, t0, NT, "qA") for j in range(3)]
        gL3 = [gather_j(Tg3[:, :], 128, wL3, N1, j, t0, NT, "qC") for j in range(3)]
        gL4 = [gather_j(Tg4[:, :], 256, wL4, N1, j, t0, NT, "qB") for j in range(3)]
        gL5 = [gather_j(Tg5[:, :], 256, wL5, N1, j, t0, NT, "qD") for j in range(3)]
        for t in range(t0, t0 + NT):
            a2 = wsum(gL2, "L2", t, t0, 128)
            a3 = wsum(gL3, "L3", t, t0, 128)
            a4 = wsum(gL4, "L4", t, t0, 256)
            a5 = wsum(gL5, "L5", t, t0, 256)
            fc = [fcatp.tile([128, 128], F32, tag="fc%d" % k, name="fc%d" % k)
                  for k in range(6)]
            transpose_to(fc[0][:], a4[:, 0:128], 128)
            transpose_to(fc[1][:], a4[:, 128:256], 128)
            transpose_to(fc[2][:], a5[:, 0:128], 128)
            transpose_to(fc[3][:], a5[:, 128:256], 128)
            transpose_to(fc[4][:], a3[:], 128)
            d1in = fcatp.tile([128, 128], F32, tag="d1in")
            pta = psum.tile([128, 128], F32, tag="tr")
            nc.tensor.transpose(pta[0:64, 0:128], a2[:, 0:64], ident[:])
            nc.scalar.copy(d1in[0:64, :], pta[0:64, :])
            ptb = psum.tile([128, 128], F32, tag="tr")
            nc.tensor.transpose(ptb[0:64, 0:128], a2[:, 64:128], ident[:])
            nc.scalar.copy(fc[5][0:64, :], ptb[0:64, :])
            nc.sync.dma_start(
                out=d1in[64:96, :],
                in_=ins["f1"][:, t * 128 : (t + 1) * 128],
            )
            h1 = conv([(d1in[0:96, :], 96)], "Wdec1a", "Sdec1a", 32, 128)
            conv([(h1[0][0:32, 0:128], 32)], "Wdec1b", "Sdec1b", 32, 128,
                 out_into=[fc[5][64:96, :]], pbase=64)
            xs = [(fc[k][:], 128) for k in range(5)] + [(fc[5][0:96, :], 96)]
            hf = conv(xs, "Wfusea", "Sfusea", 64, 128)
            obuf = fcatp.tile([32, 128], F32, tag="obuf", name="obuf")
            conv([(hf[0][0:64, 0:128], 64)], "Wfuseb", "Sfuseb", 32, 128,
                 out_into=[obuf[0:32, 0:128]])
            nc.sync.dma_start(
                out=out_h[:, t * 128 : (t + 1) * 128], in_=obuf[:]
            )


# ------------------------------- entry -------------------------------------

_NC_CACHE = {}


def _get_nc():
    if "nc" not in _NC_CACHE:
        _NC_CACHE["nc"] = build_kernel()
    return _NC_CACHE["nc"]


def kernel(**inputs):
    nc = _get_nc()
    in_maps = [host_prep(inputs, b) for b in range(B)]
    res = run_bass_kernel_spmd(nc, in_maps, list(range(B)))
    out = np.stack([res.results[b]["out"] for b in range(B)], axis=0)
    return out.astype(np.float32)


if __name__ == "__main__":
    out = None
    import json
    print("build only")
    nc = _get_nc()
    print("built ok")
